# revision 18
# baseline (speedup 1.0000x reference)
"""Trainium2 Bass kernel for nn_IntensityLoss (bilateral-filter intensity loss).

Math (all window sums use raw r_weights; the 1/25 normalizations cancel):
  A  = sum_t w_t                (25-tap sum, per pixel)
  Bf = sum_t fake_t  w_t ; Cf = sum_t fake_t^2  w_t   (taps = 5x5 shifted copies)
  Bg, Cg  likewise for gamma_hdr
  Bh = sum_t H_t w_t  with  H = hdr_original_im ** (1 - f)   (zero-padded)
  Vx  = relu(Cx*A - Bx^2 + eps*A^2)          (~= A^2 * (var + eps))
  num = K * sqrt(Vg) * (Bh + eps*A)          (K = gray_max / f)
  den = A * sqrt(Vf) + num
  r   = num / den                            (= 1 - std_fake/(std_fake+std_obj))
  out = sum(r * (A-1)) / sum(A-1)            (global over B*H*W pixels)

Sharding: core c handles batch b=c//2, rows [256*(c%2), +256).  Each core pads
to 275 "virtual" rows (11 chunks x 25 rows); pad rows get w=0.04 so A~=1.

Layout: "diagonal stack" [125 partitions = 5 row-shifts x 25 rows, 512 cols].
All inputs host-cast to bf16.

Measured engine facts this schedule is built on: warm matmul cadence is
215 ns (PE does a chunk in 6.5 us); DVE 4D fused bf16 muls hit 2x mode
(1.75 us per f+g tap-group); GpSimd is terrible at tensor ops (3-13
cyc/elem) but fine at SWDGE descriptor generation; both HWDGE rings share
SDMA engines 64-68 while SWDGE spreads over 69-79.

Engine roles:
  sync   - HWDGE chunk DMAs (h0, fg0)
  gpsimd - SWDGE bulk DMAs for chunk c+1 (wt, fg1, h1); no compute
  DVE    - ALL window products (fused 4D, 2x) + batched tail epilogue
  PE     - selector matmuls (A-stat first; it only needs the weight DMA)
  ScalarE- prologue, per-g PSUM evacuation + mean-squares, tail relu/sqrt
Stats from all 3 super-chunks accumulate into one [125, 3, 4, 512] buffer;
the entire epilogue then runs once, batched over 3*512 columns.
"""

import sys

sys.path.insert(0, "/opt/trn_rl_repo")

import numpy as np
import ml_dtypes

import concourse.bass as bass
import concourse.bacc as bacc
import concourse.tile as tile
from concourse import mybir
from concourse.bass_utils import run_bass_kernel_spmd

F32 = mybir.dt.float32
BF16 = mybir.dt.bfloat16
AF = mybir.ActivationFunctionType
ALU = mybir.AluOpType
AX = mybir.AxisListType

EPS = 1e-5
H_IMG = 512
W_IMG = 512
B_SZ = 4
N_CORES = 8
RPC = 256
QR = 25
NCH = 11
VROWS = NCH * QR   # 275
PROWS = 280
PCOLS = 516
FGC = 2 * PCOLS    # 1032
WPAD = 0.04

_CACHE = {}


def _build_nc():
    nc = bacc.Bacc(None)
    wslab = nc.declare_dram_parameter("wslab", [5, VROWS, 5, W_IMG], BF16, isOutput=False)
    imfg = nc.declare_dram_parameter("imfg", [PROWS, 2, PCOLS], BF16, isOutput=False)
    imh = nc.declare_dram_parameter("imh", [PROWS, PCOLS], BF16, isOutput=False)
    hmask = nc.declare_dram_parameter("hmask", [PROWS, 1], F32, isOutput=False)
    gray = nc.declare_dram_parameter("gray", [H_IMG, W_IMG], BF16, isOutput=False)
    scal = nc.declare_dram_parameter("scal", [1, 4], F32, isOutput=False)
    stat = nc.declare_dram_parameter("stat", [5, 125, 125], BF16, isOutput=False)
    out = nc.declare_dram_parameter("out", [125, 2], F32, isOutput=True)

    himg = nc.dram_tensor("himg", [PROWS, PCOLS], BF16)

    HWT = VROWS * 5 * W_IMG

    with tile.TileContext(nc) as tc:
        with (
            tc.tile_pool(name="singles", bufs=1) as singles,
            tc.tile_pool(name="prep", bufs=2) as prep,
            tc.tile_pool(name="chunk", bufs=3) as chunk,
            tc.tile_pool(name="prod", bufs=3) as prod,
            tc.tile_pool(name="epi", bufs=1) as epi,
            tc.tile_pool(name="psAB", bufs=1, space="PSUM") as psum_stats,
            tc.tile_pool(name="psM", bufs=1, space="PSUM") as psum_misc,
        ):
            # ---------- phase 0 ----------
            st_all = singles.tile([125, 5, 125], BF16)
            nc.sync.dma_start(
                out=st_all[:],
                in_=bass.AP(
                    tensor=stat, offset=0,
                    ap=[[125, 125], [125 * 125, 5], [1, 125]],
                ),
            )
            sc = singles.tile([1, 4], F32)
            nc.sync.dma_start(out=sc[:], in_=scal[:])

            ones = singles.tile([1, 128], F32)
            nc.vector.memset(ones[:], 1.0)

            fb = singles.tile([128, 2], F32)
            ps_bc = psum_misc.tile([128, 2], F32, tag="bc")
            nc.tensor.matmul(ps_bc[:], ones[:], sc[0:1, 0:2], start=True, stop=True)
            nc.scalar.copy(fb[:], ps_bc[:])
            f1m_bc = fb[:, 0:1]
            finv_bc = fb[:, 1:2]

            H_TILES = [(0, 128), (128, 128), (256, PROWS - 256)]
            row_tiles = []
            def emit_h_tiles():
              for r0, p in H_TILES:
                ht = prep.tile([128, PCOLS], BF16, tag="ht")
                nc.sync.dma_start(out=ht[:p, :], in_=imh[r0 : r0 + p, :])
                lt = prep.tile([128, PCOLS], F32, tag="lt")
                nc.scalar.activation(lt[:p, :], ht[:p, :], AF.Ln)
                et0 = prep.tile([128, PCOLS], BF16, tag="et0")
                nc.scalar.activation(et0[:p, :], lt[:p, :], AF.Exp, scale=f1m_bc[:p, :])
                hm = prep.tile([128, 1], F32, tag="hm")
                nc.sync.dma_start(out=hm[:p, :], in_=hmask[r0 : r0 + p, :])
                et = prep.tile([128, PCOLS], BF16, tag="et")
                nc.scalar.activation(et[:p, :], et0[:p, :], AF.Copy,
                                     scale=hm[:p, 0:1])
                nc.scalar.memzero(et[:p, 0:2])
                nc.scalar.memzero(et[:p, 514:516])
                nc.sync.dma_start(out=himg[r0 : r0 + p, :], in_=et[:p, :])

            gt = prep.tile([128, 2048], BF16)

            accC01 = singles.tile([125, 1], F32)
            accC2 = singles.tile([125, 1], F32)
            accA = [singles.tile([125, 1], F32, name=f"accA{i}") for i in range(3)]
            gm = singles.tile([128, 1], F32)
            gmr = singles.tile([1, 128], F32)
            gms = singles.tile([1, 1], F32)
            kk = singles.tile([128, 2], F32)
            k2_sb = kk[:, 1:2]

            # stats for all 3 super-chunks: [125, g, slot(A,Cf,Cg,Bh), 512]
            gstA = singles.tile([125, 3, 4, 512], BF16)
            t2A = singles.tile([125, 3, 512], BF16)   # Bf^2 per g
            t3A = singles.tile([125, 3, 512], BF16)   # Bg^2 per g

            psA = None
            psB = None
            swd = {}


            def emit_tail(g_lo, ng, acc):
                """Epilogue for super-chunks [g_lo, g_lo+ng) batched over ng*512 cols."""
                GT = gstA.tensor
                base = g_lo * 2048

                def gs(slot):
                    return bass.AP(tensor=GT, offset=base + slot * 512,
                                   ap=[[6144, 125], [2048, ng], [1, 512]])

                A3, Cf3, Cg3, Bh3 = gs(0), gs(1), gs(2), gs(3)
                t2s = t2A[:, g_lo : g_lo + ng, :]
                t3s = t3A[:, g_lo : g_lo + ng, :]

                def et3(tag, dt=BF16):
                    return epi.tile([125, ng, 512], dt, tag=tag,
                                    name=f"{tag}{g_lo}", padded_shape=[125, 2, 512])

                e = et3("e")
                nc.scalar.activation(e[:], A3, AF.Square, scale=float(np.sqrt(EPS)))
                ea = et3("ea")
                nc.scalar.activation(ea[:], A3, AF.Copy, scale=EPS)

                vg = et3("vg")
                nc.vector.tensor_mul(vg[:], Cg3, A3)
                nc.vector.tensor_tensor(vg[:], vg[:], t3s, op=ALU.subtract)
                nc.vector.tensor_add(vg[:], vg[:], e[:])
                sg = et3("sg")
                nc.scalar.activation(sg[:], vg[:], AF.Relu)
                nc.scalar.activation(vg[:], sg[:], AF.Sqrt, scale=k2_sb[:125, :])
                bh2 = et3("bh2")
                nc.vector.tensor_add(bh2[:], ea[:], Bh3)
                nc.vector.tensor_mul(bh2[:], bh2[:], vg[:])   # bh2 -> num

                vf = et3("vf")
                nc.vector.tensor_mul(vf[:], Cf3, A3)
                nc.vector.tensor_tensor(vf[:], vf[:], t2s, op=ALU.subtract)
                nc.vector.tensor_add(vf[:], vf[:], e[:])
                nc.scalar.activation(sg[:], vf[:], AF.Relu)
                nc.scalar.activation(vf[:], sg[:], AF.Sqrt)

                den0 = et3("den0", F32)
                nc.vector.tensor_mul(den0[:], A3, vf[:])
                nc.vector.scalar_tensor_tensor(
                    den0[:], den0[:], 1e-30, bh2[:], op0=ALU.add, op1=ALU.add
                )
                rden = et3("rden", F32)
                pstride = den0.ap[0][0]
                den_f = bass.AP(tensor=den0.tensor, offset=0,
                                ap=[[pstride, 125], [1, ng * 512]])
                rden_f = bass.AP(tensor=rden.tensor, offset=0,
                                 ap=[[pstride, 125], [1, ng * 512]])
                nc.vector.reciprocal_approx_fast(rden_f, den_f)
                nc.vector.tensor_mul(bh2[:], bh2[:], rden[:])  # bh2 -> r
                cb = et3("cb")
                nc.vector.scalar_tensor_tensor(
                    cb[:], A3, -1.0, bh2[:],
                    op0=ALU.add, op1=ALU.mult, accum_out=acc[:],
                )

            def issue_swdge(c):
                cr0 = c * QR
                eng = nc.sync if c == 0 else nc.gpsimd
                wt = prod.tile([125, 5, 512], BF16, tag="wt", name=f"wt{c}",
                               bufs=5)
                eng.dma_start(
                    out=wt[:],
                    in_=bass.AP(
                        tensor=wslab, offset=cr0 * 5 * W_IMG,
                        ap=[[HWT, 5], [5 * W_IMG, QR], [1, 5 * W_IMG]],
                    ),
                )
                fg1 = chunk.tile([125, 1036], BF16, tag="fg1", name=f"fg1_{c}",
                                 bufs=5)
                eng.dma_start(
                    out=fg1[:, 0:1031],
                    in_=bass.AP(
                        tensor=imfg, offset=cr0 * FGC + 1,
                        ap=[[FGC, 5], [FGC, QR], [1, FGC - 1]],
                    ),
                )
                h1 = chunk.tile([125, PCOLS], BF16, tag="h1", name=f"h1_{c}",
                                bufs=5)
                eng.dma_start(
                    out=h1[:, 0:515],
                    in_=bass.AP(
                        tensor=himg, offset=cr0 * PCOLS + 1,
                        ap=[[PCOLS, 5], [PCOLS, QR], [1, PCOLS - 1]],
                    ),
                )
                swd[c] = (wt, fg1, h1)

            fg0_pre = {}

            def issue_fg0(c, eng):
                fg0 = chunk.tile([125, 2, PCOLS], BF16, tag="fg0", name=f"fg0_{c}")
                eng.dma_start(
                    out=fg0[:],
                    in_=bass.AP(
                        tensor=imfg, offset=c * QR * FGC,
                        ap=[[FGC, 5], [FGC, QR], [1, FGC]],
                    ),
                )
                fg0_pre[c] = fg0

            issue_swdge(0)
            issue_fg0(0, nc.sync)
            emit_h_tiles()
            issue_swdge(1)

            # ---------- phase 1: chunks ----------
            for c in range(NCH):
                s = c % 5
                g = c // 5
                last_s = 4 if g < 2 else 0
                cr0 = c * QR

                wt, fg1, h1 = swd.pop(c)
                Pfg1 = prod.tile([125, 2, 5, 512], BF16, tag="p1", name=f"p1_{c}")
                Pfg2 = prod.tile([125, 2, 5, 512], BF16, tag="p2", name=f"p2_{c}")
                Ph = prod.tile([125, 5, 512], BF16, tag="ph", name=f"ph{c}")

                fg0 = fg0_pre.pop(c)
                h0 = chunk.tile([125, PCOLS], BF16, tag="h0")
                nc.sync.dma_start(
                    out=h0[:],
                    in_=bass.AP(
                        tensor=himg, offset=cr0 * PCOLS,
                        ap=[[PCOLS, 5], [PCOLS, QR], [1, PCOLS]],
                    ),
                )

                if c + 1 < NCH:
                    issue_fg0(c + 1, nc.sync)
                if c + 2 < NCH:
                    issue_swdge(c + 2)
                if c == 0:
                    nc.gpsimd.dma_start(
                        out=gt[:],
                        in_=bass.AP(tensor=gray, offset=0,
                                    ap=[[2048, 128], [1, 2048]]),
                    )

                P1t, P2t = Pfg1.tensor, Pfg2.tensor
                p1_out_e = bass.AP(tensor=P1t, offset=0,
                                   ap=[[5120, 125], [2560, 2], [1024, 3], [1, 512]])
                p1_out_o = bass.AP(tensor=P1t, offset=512,
                                   ap=[[5120, 125], [2560, 2], [1024, 2], [1, 512]])
                p2_out_e = bass.AP(tensor=P2t, offset=0,
                                   ap=[[5120, 125], [2560, 2], [1024, 3], [1, 512]])
                p2_out_o = bass.AP(tensor=P2t, offset=512,
                                   ap=[[5120, 125], [2560, 2], [1024, 2], [1, 512]])
                fg0_w = bass.AP(tensor=fg0.tensor, offset=0,
                                ap=[[FGC, 125], [PCOLS, 2], [2, 3], [1, 512]])
                fg1_w = bass.AP(tensor=fg1.tensor, offset=0,
                                ap=[[1036, 125], [PCOLS, 2], [2, 2], [1, 512]])
                wt_bc_e = bass.AP(tensor=wt.tensor, offset=0,
                                  ap=[[2560, 125], [0, 2], [1024, 3], [1, 512]])
                wt_bc_o = bass.AP(tensor=wt.tensor, offset=512,
                                  ap=[[2560, 125], [0, 2], [1024, 2], [1, 512]])
                h0w = bass.AP(tensor=h0.tensor, offset=0,
                              ap=[[PCOLS, 125], [2, 3], [1, 512]])
                h1w = bass.AP(tensor=h1.tensor, offset=0,
                              ap=[[PCOLS, 125], [2, 2], [1, 512]])

                nc.vector.tensor_mul(p1_out_e, fg0_w, wt_bc_e)
                nc.vector.tensor_mul(Ph[:, 0:5:2, :], h0w, wt[:, 0:5:2, :])
                nc.vector.tensor_mul(p1_out_o, fg1_w, wt_bc_o)
                nc.vector.tensor_mul(p2_out_e, p1_out_e, fg0_w)
                nc.vector.tensor_mul(p2_out_o, p1_out_o, fg1_w)
                nc.vector.tensor_mul(Ph[:, 1:5:2, :], h1w, wt[:, 1:5:2, :])

                if c == NCH - 1:
                    nc.vector.tensor_reduce(gm[:], gt[:], axis=AX.X, op=ALU.max)
                    nc.sync.dma_start(out=gmr[:], in_=gm[:])
                    nc.vector.tensor_reduce(gms[:], gmr[:], axis=AX.X, op=ALU.max)
                    ps_bc3 = psum_misc.tile([128, 1], F32, tag="bc3")
                    nc.tensor.matmul(ps_bc3[:], ones[:], gms[0:1, 0:1],
                                     start=True, stop=True)
                    gmb = prep.tile([128, 1], F32, tag="gmb")
                    nc.scalar.copy(gmb[:], ps_bc3[:])
                    nc.vector.tensor_mul(kk[:, 0:1], gmb[:], finv_bc)
                    nc.vector.tensor_mul(kk[:, 1:2], kk[:, 0:1], kk[:, 0:1])
                    emit_tail(0, 2, accC01)

                if s == 0:
                    psA = psum_stats.tile([125, 3, 512], F32, tag="psA", name="psA")
                    psB = psum_stats.tile([125, 3, 512], F32, tag="psB", name="psB")
                border = [0, 2, 4, 1, 3]
                srcs = {
                    0: (wt.tensor, 0, 2560), 1: (P1t, 0, 5120),
                    2: (P2t, 0, 5120), 3: (P1t, 2560, 5120),
                    4: (P2t, 2560, 5120), 5: (Ph.tensor, 0, 2560),
                }
                if s == last_s:
                    stat_order = [(0, psA, 0), (1, psA, 1), (2, psA, 2),
                                  (3, psB, 0), (4, psB, 1), (5, psB, 2)]
                else:
                    stat_order = [(0, psA, 0), (1, psA, 1), (3, psB, 0),
                                  (2, psA, 2), (4, psB, 1), (5, psB, 2)]
                for slot, ps, j in stat_order:
                    mt, mbase, mstride = srcs[slot]
                    for i, b in enumerate(border):
                        mov = bass.AP(
                            tensor=mt,
                            offset=mbase + b * 512,
                            ap=[[mstride, 125], [1, 512]],
                        )
                        nc.tensor.matmul(
                            ps[:, j, :],
                            st_all[:, s, :],
                            mov,
                            start=(s == 0 and i == 0),
                            stop=(s == last_s and i == 4),
                        )

                if s == last_s:
                    # PSUM evacuation into the batched stat buffers
                    nc.scalar.activation(
                        gstA[:, g, 0, :], psA[:, 0, :], AF.Copy,
                        accum_out=accA[g][:],
                    )
                    nc.scalar.square(t2A[:, g, :], psA[:, 1, :])
                    nc.scalar.copy(gstA[:, g, 1, :], psA[:, 2, :])
                    nc.scalar.square(t3A[:, g, :], psB[:, 0, :])
                    nc.scalar.copy(gstA[:, g, 2:4, :], psB[:, 1:3, :])

            emit_tail(2, 1, accC2)

            # ---------- final reduce ----------
            red = singles.tile([125, 2], F32)
            nc.vector.tensor_add(red[:, 0:1], accC01[:], accC2[:])
            nc.vector.tensor_add(red[:, 1:2], accA[0][:], accA[1][:])
            nc.vector.tensor_add(red[:, 1:2], red[:, 1:2], accA[2][:])
            nc.sync.dma_start(out=out[:], in_=red[:])

    nc.compile()
    return nc


def _host_inputs(fake, gamma_hdr, hdr_original_im, r_weights, f_factors,
                 hdr_original_gray):
    """Build the 8 per-core input dicts (all image data host-cast to bf16)."""
    bf16 = ml_dtypes.bfloat16
    stat_np = np.zeros((5, 125, 125), dtype=np.float32)
    for s in range(5):
        for a in range(5):
            for q in range(25):
                stat_np[s, a * 25 + q, s * 25 + q] = 1.0
    stat_np = stat_np.astype(bf16)

    def padimg(x, cval):
        return np.pad(x, ((2, 22), (2, 2)), constant_values=cval).astype(
            np.float32
        )

    in_maps = []
    for c in range(N_CORES):
        b = c // 2
        r0 = (c % 2) * RPC
        slab = np.full((5, 5, VROWS, W_IMG), WPAD, dtype=np.float32)
        slab[:, :, :RPC, :] = r_weights[b, :, r0 : r0 + RPC, :].reshape(
            5, 5, RPC, W_IMG
        )
        slab = np.ascontiguousarray(slab.transpose(0, 2, 1, 3))

        pf = padimg(fake[b, 0], 0.0)[r0 : r0 + PROWS]
        pg = padimg(gamma_hdr[b, 0], 0.0)[r0 : r0 + PROWS]
        imfg = np.ascontiguousarray(np.stack([pf, pg], axis=1).astype(bf16))
        ph = padimg(hdr_original_im[b, 0], 1.0)[r0 : r0 + PROWS]
        gidx = r0 + np.arange(PROWS)
        hm = ((gidx >= 2) & (gidx <= 513)).astype(np.float32).reshape(PROWS, 1)

        f = float(f_factors[b])
        scal = np.array([[1.0 - f, 1.0 / f, 0.0, 0.0]], dtype=np.float32)

        in_maps.append(
            {
                "wslab": slab.astype(bf16),
                "imfg": imfg,
                "imh": np.ascontiguousarray(ph).astype(bf16),
                "hmask": hm,
                "gray": np.ascontiguousarray(hdr_original_gray[b, 0]).astype(bf16),
                "scal": scal,
                "stat": stat_np,
            }
        )
    return in_maps


def kernel_run(inputs, **spmd_kwargs):
    """Returns (scalar_result, BassKernelResults)."""
    if "nc" not in _CACHE:
        _CACHE["nc"] = _build_nc()
    nc = _CACHE["nc"]
    in_maps = _host_inputs(**inputs)
    res = run_bass_kernel_spmd(nc, in_maps, list(range(N_CORES)), **spmd_kwargs)
    s1 = 0.0
    s2 = 0.0
    for r in res.results:
        o = np.asarray(r["out"], dtype=np.float64)
        s1 += o[:, 0].sum()
        s2 += o[:, 1].sum() - 512.0 * VROWS
    return np.float32(s1 / s2), res


def kernel(**inputs):
    result, _ = kernel_run(inputs)
    return result


# revision 21
# speedup vs baseline: 1.0425x; 1.0425x over previous
"""Trainium2 Bass kernel for nn_IntensityLoss (bilateral-filter intensity loss).

Math (all window sums use raw r_weights; the 1/25 normalizations cancel):
  A  = sum_t w_t                (25-tap sum, per pixel)
  Bf = sum_t fake_t  w_t ; Cf = sum_t fake_t^2  w_t   (taps = 5x5 shifted copies)
  Bg, Cg  likewise for gamma_hdr
  Bh = sum_t H_t w_t  with  H = hdr_original_im ** (1 - f)   (zero-padded)
  Vx  = relu(Cx*A - Bx^2 + eps*A^2)          (~= A^2 * (var + eps))
  num = K * sqrt(Vg) * (Bh + eps*A)          (K = gray_max / f)
  den = A * sqrt(Vf) + num
  r   = num / den                            (= 1 - std_fake/(std_fake+std_obj))
  out = sum(r * (A-1)) / sum(A-1)            (global over B*H*W pixels)

Sharding: core c handles batch b=c//2, rows [256*(c%2), +256).  Each core pads
to 275 "virtual" rows (11 chunks x 25 rows); pad rows get w=0.04 so A~=1.

Layout: "diagonal stack" [125 partitions = 5 row-shifts x 25 rows, 512 cols].
All inputs host-cast to bf16.

Measured engine facts this schedule is built on: warm matmul cadence is
215 ns (PE does a chunk in 6.5 us); DVE 4D fused bf16 muls hit 2x mode
(1.75 us per f+g tap-group); GpSimd is terrible at tensor ops (3-13
cyc/elem) but fine at SWDGE descriptor generation; both HWDGE rings share
SDMA engines 64-68 while SWDGE spreads over 69-79.

Engine roles:
  sync   - HWDGE chunk DMAs (h0, fg0)
  gpsimd - SWDGE bulk DMAs for chunk c+1 (wt, fg1, h1); no compute
  DVE    - ALL window products (fused 4D, 2x) + batched tail epilogue
  PE     - selector matmuls (A-stat first; it only needs the weight DMA)
  ScalarE- prologue, per-g PSUM evacuation + mean-squares, tail relu/sqrt
Stats from all 3 super-chunks accumulate into one [125, 3, 4, 512] buffer;
the entire epilogue then runs once, batched over 3*512 columns.
"""

import sys

sys.path.insert(0, "/opt/trn_rl_repo")

import numpy as np
import ml_dtypes

import concourse.bass as bass
import concourse.bacc as bacc
import concourse.tile as tile
from concourse import mybir
from concourse.bass_utils import run_bass_kernel_spmd

F32 = mybir.dt.float32
BF16 = mybir.dt.bfloat16
AF = mybir.ActivationFunctionType
ALU = mybir.AluOpType
AX = mybir.AxisListType

EPS = 1e-5
H_IMG = 512
W_IMG = 512
B_SZ = 4
N_CORES = 8
RPC = 256
QR = 25
NCH = 11
VROWS = NCH * QR   # 275
PROWS = 280
PCOLS = 516
FGC = 2 * PCOLS    # 1032
WPAD = 0.04

_CACHE = {}


def _build_nc():
    nc = bacc.Bacc(None)
    wslab = nc.declare_dram_parameter("wslab", [5, VROWS, 5, W_IMG], BF16, isOutput=False)
    imfg = nc.declare_dram_parameter("imfg", [PROWS, 2, PCOLS], BF16, isOutput=False)
    imh = nc.declare_dram_parameter("imh", [PROWS, PCOLS], BF16, isOutput=False)
    hmask = nc.declare_dram_parameter("hmask", [PROWS, 1], F32, isOutput=False)
    gray = nc.declare_dram_parameter("gray", [H_IMG, W_IMG], BF16, isOutput=False)
    scal = nc.declare_dram_parameter("scal", [1, 4], F32, isOutput=False)
    stat = nc.declare_dram_parameter("stat", [5, 125, 125], BF16, isOutput=False)
    out = nc.declare_dram_parameter("out", [125, 2], F32, isOutput=True)

    himg = nc.dram_tensor("himg", [PROWS, PCOLS], BF16)

    HWT = VROWS * 5 * W_IMG

    with tile.TileContext(nc) as tc:
        with (
            tc.tile_pool(name="singles", bufs=1) as singles,
            tc.tile_pool(name="prep", bufs=2) as prep,
            tc.tile_pool(name="chunk", bufs=3) as chunk,
            tc.tile_pool(name="prod", bufs=3) as prod,
            tc.tile_pool(name="epi", bufs=1) as epi,
            tc.tile_pool(name="psAB", bufs=1, space="PSUM") as psum_stats,
            tc.tile_pool(name="psM", bufs=1, space="PSUM") as psum_misc,
        ):
            # ---------- phase 0 ----------
            st_all = singles.tile([125, 5, 125], BF16)
            nc.sync.dma_start(
                out=st_all[:],
                in_=bass.AP(
                    tensor=stat, offset=0,
                    ap=[[125, 125], [125 * 125, 5], [1, 125]],
                ),
            )
            sc = singles.tile([1, 4], F32)
            nc.sync.dma_start(out=sc[:], in_=scal[:])

            ones = singles.tile([1, 128], F32)
            nc.vector.memset(ones[:], 1.0)

            fb = singles.tile([128, 2], F32)
            ps_bc = psum_misc.tile([128, 2], F32, tag="bc")
            nc.tensor.matmul(ps_bc[:], ones[:], sc[0:1, 0:2], start=True, stop=True)
            nc.scalar.copy(fb[:], ps_bc[:])
            f1m_bc = fb[:, 0:1]
            finv_bc = fb[:, 1:2]

            H_TILES = [(0, 128), (128, 128), (256, PROWS - 256)]
            row_tiles = []
            def emit_h_tiles():
              for r0, p in H_TILES:
                ht = prep.tile([128, PCOLS], BF16, tag="ht")
                nc.sync.dma_start(out=ht[:p, :], in_=imh[r0 : r0 + p, :])
                lt = prep.tile([128, PCOLS], F32, tag="lt")
                nc.scalar.activation(lt[:p, :], ht[:p, :], AF.Ln)
                et0 = prep.tile([128, PCOLS], BF16, tag="et0")
                nc.scalar.activation(et0[:p, :], lt[:p, :], AF.Exp, scale=f1m_bc[:p, :])
                hm = prep.tile([128, 1], F32, tag="hm")
                nc.sync.dma_start(out=hm[:p, :], in_=hmask[r0 : r0 + p, :])
                et = prep.tile([128, PCOLS], BF16, tag="et")
                nc.scalar.activation(et[:p, :], et0[:p, :], AF.Copy,
                                     scale=hm[:p, 0:1])
                nc.scalar.memzero(et[:p, 0:2])
                nc.scalar.memzero(et[:p, 514:516])
                nc.sync.dma_start(out=himg[r0 : r0 + p, :], in_=et[:p, :])

            gt = prep.tile([128, 2048], BF16)

            accC01 = singles.tile([125, 1], F32)
            accC2 = singles.tile([125, 1], F32)
            accA = [singles.tile([125, 1], F32, name=f"accA{i}") for i in range(3)]
            gm = singles.tile([128, 1], F32)
            gmr = singles.tile([1, 128], F32)
            gms = singles.tile([1, 1], F32)
            kk = singles.tile([128, 2], F32)
            k2_sb = kk[:, 1:2]

            # stats for all 3 super-chunks: [125, g, slot(A,Cf,Cg,Bh), 512]
            gstA = singles.tile([125, 3, 4, 512], BF16)
            t2A = singles.tile([125, 3, 512], BF16)   # Bf^2 per g
            t3A = singles.tile([125, 3, 512], BF16)   # Bg^2 per g

            psA = None
            psB = None
            swd = {}


            def emit_tail(g_lo, ng, acc):
                """Epilogue for super-chunks [g_lo, g_lo+ng) batched over ng*512 cols."""
                GT = gstA.tensor
                base = g_lo * 2048

                def gs(slot):
                    return bass.AP(tensor=GT, offset=base + slot * 512,
                                   ap=[[6144, 125], [2048, ng], [1, 512]])

                A3, Cf3, Cg3, Bh3 = gs(0), gs(1), gs(2), gs(3)
                t2s = t2A[:, g_lo : g_lo + ng, :]
                t3s = t3A[:, g_lo : g_lo + ng, :]

                def et3(tag, dt=BF16):
                    return epi.tile([125, ng, 512], dt, tag=tag,
                                    name=f"{tag}{g_lo}", padded_shape=[125, 2, 512])

                e = et3("e")
                nc.scalar.activation(e[:], A3, AF.Square, scale=float(np.sqrt(EPS)))
                ea = et3("ea")
                nc.scalar.activation(ea[:], A3, AF.Copy, scale=EPS)

                vg = et3("vg")
                nc.vector.tensor_mul(vg[:], Cg3, A3)
                nc.vector.tensor_tensor(vg[:], vg[:], t3s, op=ALU.subtract)
                nc.vector.tensor_add(vg[:], vg[:], e[:])
                sg = et3("sg")
                nc.scalar.activation(sg[:], vg[:], AF.Relu)
                nc.scalar.activation(vg[:], sg[:], AF.Sqrt, scale=k2_sb[:125, :])
                bh2 = et3("bh2")
                nc.vector.tensor_add(bh2[:], ea[:], Bh3)
                nc.vector.tensor_mul(bh2[:], bh2[:], vg[:])   # bh2 -> num

                vf = et3("vf")
                nc.vector.tensor_mul(vf[:], Cf3, A3)
                nc.vector.tensor_tensor(vf[:], vf[:], t2s, op=ALU.subtract)
                nc.vector.tensor_add(vf[:], vf[:], e[:])
                nc.scalar.activation(sg[:], vf[:], AF.Relu)
                nc.scalar.activation(vf[:], sg[:], AF.Sqrt)

                den0 = et3("den0", F32)
                nc.vector.tensor_mul(den0[:], A3, vf[:])
                nc.vector.scalar_tensor_tensor(
                    den0[:], den0[:], 1e-30, bh2[:], op0=ALU.add, op1=ALU.add
                )
                rden = et3("rden", F32)
                pstride = den0.ap[0][0]
                den_f = bass.AP(tensor=den0.tensor, offset=0,
                                ap=[[pstride, 125], [1, ng * 512]])
                rden_f = bass.AP(tensor=rden.tensor, offset=0,
                                 ap=[[pstride, 125], [1, ng * 512]])
                nc.vector.reciprocal_approx_fast(rden_f, den_f)
                nc.vector.tensor_mul(bh2[:], bh2[:], rden[:])  # bh2 -> r
                cb = et3("cb")
                nc.vector.scalar_tensor_tensor(
                    cb[:], A3, -1.0, bh2[:],
                    op0=ALU.add, op1=ALU.mult, accum_out=acc[:],
                )

            def issue_swdge(c, skip_h1=False):
                cr0 = c * QR
                eng = nc.sync if c == 0 else nc.gpsimd
                wt = prod.tile([125, 5, 512], BF16, tag="wt", name=f"wt{c}")
                eng.dma_start(
                    out=wt[:],
                    in_=bass.AP(
                        tensor=wslab, offset=cr0 * 5 * W_IMG,
                        ap=[[HWT, 5], [5 * W_IMG, QR], [1, 5 * W_IMG]],
                    ),
                )
                fg1 = chunk.tile([125, 1036], BF16, tag="fg1", name=f"fg1_{c}")
                eng.dma_start(
                    out=fg1[:, 0:1031],
                    in_=bass.AP(
                        tensor=imfg, offset=cr0 * FGC + 1,
                        ap=[[FGC, 5], [FGC, QR], [1, FGC - 1]],
                    ),
                )
                if skip_h1:
                    swd[c] = (wt, fg1, None)
                    return
                h1 = chunk.tile([125, PCOLS], BF16, tag="h1", name=f"h1_{c}")
                eng.dma_start(
                    out=h1[:, 0:515],
                    in_=bass.AP(
                        tensor=himg, offset=cr0 * PCOLS + 1,
                        ap=[[PCOLS, 5], [PCOLS, QR], [1, PCOLS - 1]],
                    ),
                )
                swd[c] = (wt, fg1, h1)

            def issue_h1(c, eng):
                h1 = chunk.tile([125, PCOLS], BF16, tag="h1", name=f"h1b_{c}")
                eng.dma_start(
                    out=h1[:, 0:515],
                    in_=bass.AP(
                        tensor=himg, offset=c * QR * PCOLS + 1,
                        ap=[[PCOLS, 5], [PCOLS, QR], [1, PCOLS - 1]],
                    ),
                )
                wt, fg1, _ = swd[c]
                swd[c] = (wt, fg1, h1)

            fg0_pre = {}

            def issue_fg0(c, eng):
                fg0 = chunk.tile([125, 2, PCOLS], BF16, tag="fg0", name=f"fg0_{c}")
                eng.dma_start(
                    out=fg0[:],
                    in_=bass.AP(
                        tensor=imfg, offset=c * QR * FGC,
                        ap=[[FGC, 5], [FGC, QR], [1, FGC]],
                    ),
                )
                fg0_pre[c] = fg0

            issue_swdge(0, skip_h1=True)
            issue_fg0(0, nc.sync)
            emit_h_tiles()
            issue_h1(0, nc.sync)

            # ---------- phase 1: chunks ----------
            for c in range(NCH):
                s = c % 5
                g = c // 5
                last_s = 4 if g < 2 else 0
                cr0 = c * QR

                wt, fg1, h1 = swd.pop(c)
                Pfg1 = prod.tile([125, 2, 5, 512], BF16, tag="p1", name=f"p1_{c}")
                Pfg2 = prod.tile([125, 2, 5, 512], BF16, tag="p2", name=f"p2_{c}")
                Ph = prod.tile([125, 5, 512], BF16, tag="ph", name=f"ph{c}")

                fg0 = fg0_pre.pop(c)
                h0 = chunk.tile([125, PCOLS], BF16, tag="h0")
                nc.sync.dma_start(
                    out=h0[:],
                    in_=bass.AP(
                        tensor=himg, offset=cr0 * PCOLS,
                        ap=[[PCOLS, 5], [PCOLS, QR], [1, PCOLS]],
                    ),
                )

                if c + 1 < NCH:
                    issue_fg0(c + 1, nc.sync)
                if c + 1 < NCH:
                    issue_swdge(c + 1)
                if c == 0:
                    nc.gpsimd.dma_start(
                        out=gt[:],
                        in_=bass.AP(tensor=gray, offset=0,
                                    ap=[[2048, 128], [1, 2048]]),
                    )

                P1t, P2t = Pfg1.tensor, Pfg2.tensor
                p1_out_e = bass.AP(tensor=P1t, offset=0,
                                   ap=[[5120, 125], [2560, 2], [1024, 3], [1, 512]])
                p1_out_o = bass.AP(tensor=P1t, offset=512,
                                   ap=[[5120, 125], [2560, 2], [1024, 2], [1, 512]])
                p2_out_e = bass.AP(tensor=P2t, offset=0,
                                   ap=[[5120, 125], [2560, 2], [1024, 3], [1, 512]])
                p2_out_o = bass.AP(tensor=P2t, offset=512,
                                   ap=[[5120, 125], [2560, 2], [1024, 2], [1, 512]])
                fg0_w = bass.AP(tensor=fg0.tensor, offset=0,
                                ap=[[FGC, 125], [PCOLS, 2], [2, 3], [1, 512]])
                fg1_w = bass.AP(tensor=fg1.tensor, offset=0,
                                ap=[[1036, 125], [PCOLS, 2], [2, 2], [1, 512]])
                wt_bc_e = bass.AP(tensor=wt.tensor, offset=0,
                                  ap=[[2560, 125], [0, 2], [1024, 3], [1, 512]])
                wt_bc_o = bass.AP(tensor=wt.tensor, offset=512,
                                  ap=[[2560, 125], [0, 2], [1024, 2], [1, 512]])
                h0w = bass.AP(tensor=h0.tensor, offset=0,
                              ap=[[PCOLS, 125], [2, 3], [1, 512]])
                h1w = bass.AP(tensor=h1.tensor, offset=0,
                              ap=[[PCOLS, 125], [2, 2], [1, 512]])

                nc.vector.tensor_mul(p1_out_e, fg0_w, wt_bc_e)
                nc.vector.tensor_mul(Ph[:, 0:5:2, :], h0w, wt[:, 0:5:2, :])
                nc.vector.tensor_mul(p1_out_o, fg1_w, wt_bc_o)
                nc.vector.tensor_mul(p2_out_e, p1_out_e, fg0_w)
                nc.vector.tensor_mul(p2_out_o, p1_out_o, fg1_w)
                nc.vector.tensor_mul(Ph[:, 1:5:2, :], h1w, wt[:, 1:5:2, :])

                if c == NCH - 1:
                    nc.vector.tensor_reduce(gm[:], gt[:], axis=AX.X, op=ALU.max)
                    nc.sync.dma_start(out=gmr[:], in_=gm[:])
                    nc.vector.tensor_reduce(gms[:], gmr[:], axis=AX.X, op=ALU.max)
                    ps_bc3 = psum_misc.tile([128, 1], F32, tag="bc3")
                    nc.tensor.matmul(ps_bc3[:], ones[:], gms[0:1, 0:1],
                                     start=True, stop=True)
                    gmb = prep.tile([128, 1], F32, tag="gmb")
                    nc.scalar.copy(gmb[:], ps_bc3[:])
                    nc.vector.tensor_mul(kk[:, 0:1], gmb[:], finv_bc)
                    nc.vector.tensor_mul(kk[:, 1:2], kk[:, 0:1], kk[:, 0:1])
                    emit_tail(0, 2, accC01)

                if s == 0:
                    psA = psum_stats.tile([125, 3, 512], F32, tag="psA", name="psA")
                    psB = psum_stats.tile([125, 3, 512], F32, tag="psB", name="psB")
                border = [0, 2, 4, 1, 3]
                srcs = {
                    0: (wt.tensor, 0, 2560), 1: (P1t, 0, 5120),
                    2: (P2t, 0, 5120), 3: (P1t, 2560, 5120),
                    4: (P2t, 2560, 5120), 5: (Ph.tensor, 0, 2560),
                }
                if s == last_s:
                    stat_order = [(0, psA, 0), (1, psA, 1), (2, psA, 2),
                                  (3, psB, 0), (4, psB, 1), (5, psB, 2)]
                else:
                    stat_order = [(0, psA, 0), (1, psA, 1), (3, psB, 0),
                                  (2, psA, 2), (4, psB, 1), (5, psB, 2)]
                for slot, ps, j in stat_order:
                    mt, mbase, mstride = srcs[slot]
                    for i, b in enumerate(border):
                        mov = bass.AP(
                            tensor=mt,
                            offset=mbase + b * 512,
                            ap=[[mstride, 125], [1, 512]],
                        )
                        nc.tensor.matmul(
                            ps[:, j, :],
                            st_all[:, s, :],
                            mov,
                            start=(s == 0 and i == 0),
                            stop=(s == last_s and i == 4),
                        )

                if s == last_s:
                    # PSUM evacuation into the batched stat buffers
                    nc.scalar.activation(
                        gstA[:, g, 0, :], psA[:, 0, :], AF.Copy,
                        accum_out=accA[g][:],
                    )
                    nc.scalar.square(t2A[:, g, :], psA[:, 1, :])
                    nc.scalar.copy(gstA[:, g, 1, :], psA[:, 2, :])
                    nc.scalar.square(t3A[:, g, :], psB[:, 0, :])
                    nc.scalar.copy(gstA[:, g, 2:4, :], psB[:, 1:3, :])

            emit_tail(2, 1, accC2)

            # ---------- final reduce ----------
            red = singles.tile([125, 2], F32)
            nc.vector.tensor_add(red[:, 0:1], accC01[:], accC2[:])
            nc.vector.tensor_add(red[:, 1:2], accA[0][:], accA[1][:])
            nc.vector.tensor_add(red[:, 1:2], red[:, 1:2], accA[2][:])
            nc.sync.dma_start(out=out[:], in_=red[:])

    nc.compile()
    return nc


def _host_inputs(fake, gamma_hdr, hdr_original_im, r_weights, f_factors,
                 hdr_original_gray):
    """Build the 8 per-core input dicts (all image data host-cast to bf16)."""
    bf16 = ml_dtypes.bfloat16
    stat_np = np.zeros((5, 125, 125), dtype=np.float32)
    for s in range(5):
        for a in range(5):
            for q in range(25):
                stat_np[s, a * 25 + q, s * 25 + q] = 1.0
    stat_np = stat_np.astype(bf16)

    def padimg(x, cval):
        return np.pad(x, ((2, 22), (2, 2)), constant_values=cval).astype(
            np.float32
        )

    in_maps = []
    for c in range(N_CORES):
        b = c // 2
        r0 = (c % 2) * RPC
        slab = np.full((5, 5, VROWS, W_IMG), WPAD, dtype=np.float32)
        slab[:, :, :RPC, :] = r_weights[b, :, r0 : r0 + RPC, :].reshape(
            5, 5, RPC, W_IMG
        )
        slab = np.ascontiguousarray(slab.transpose(0, 2, 1, 3))

        pf = padimg(fake[b, 0], 0.0)[r0 : r0 + PROWS]
        pg = padimg(gamma_hdr[b, 0], 0.0)[r0 : r0 + PROWS]
        imfg = np.ascontiguousarray(np.stack([pf, pg], axis=1).astype(bf16))
        ph = padimg(hdr_original_im[b, 0], 1.0)[r0 : r0 + PROWS]
        gidx = r0 + np.arange(PROWS)
        hm = ((gidx >= 2) & (gidx <= 513)).astype(np.float32).reshape(PROWS, 1)

        f = float(f_factors[b])
        scal = np.array([[1.0 - f, 1.0 / f, 0.0, 0.0]], dtype=np.float32)

        in_maps.append(
            {
                "wslab": slab.astype(bf16),
                "imfg": imfg,
                "imh": np.ascontiguousarray(ph).astype(bf16),
                "hmask": hm,
                "gray": np.ascontiguousarray(hdr_original_gray[b, 0]).astype(bf16),
                "scal": scal,
                "stat": stat_np,
            }
        )
    return in_maps


def kernel_run(inputs, **spmd_kwargs):
    """Returns (scalar_result, BassKernelResults)."""
    if "nc" not in _CACHE:
        _CACHE["nc"] = _build_nc()
    nc = _CACHE["nc"]
    in_maps = _host_inputs(**inputs)
    res = run_bass_kernel_spmd(nc, in_maps, list(range(N_CORES)), **spmd_kwargs)
    s1 = 0.0
    s2 = 0.0
    for r in res.results:
        o = np.asarray(r["out"], dtype=np.float64)
        s1 += o[:, 0].sum()
        s2 += o[:, 1].sum() - 512.0 * VROWS
    return np.float32(s1 / s2), res


def kernel(**inputs):
    result, _ = kernel_run(inputs)
    return result
